# revision 1
# baseline (speedup 1.0000x reference)
"""Trainium2 Bass kernel for ConstrainedProbabilityMatrixFactorization.

rating = uw @ iw.T + ub + ib.T + bias + (fb_values . E[fb_indices]) @ iw.T
       = ue_aug @ rhs_aug
  with ue_aug  = [uw + offset | ub + bias | 1]   [BU, 66]
       rhs_aug = [iw.T ; ones ; ib.T]            [66, BI]

Sharding: the 1024-user batch is split across 8 NeuronCores (128 users
per core). No collectives.

The dominant cost is Q7 (SWDGE) descriptor generation for the feedback
segment-gather (~8ns/descriptor). To minimize descriptors:
  * gather from a PAIRED view of item_rating_effect_weight
    [25000, 128] (two 64-wide rows per table row) so one dma_gather
    covers everything: index = row//2 fits int16, and no second
    shard-gather is needed. Row parity is resolved by host-built
    interleaved weights w2[p, 2l+parity] = fb_values[p, l] (the other
    half-slot gets weight 0), folded into the existing DVE multiply.
  * one descriptor per (user, l) slot: 6400/core, in 2 chunked
    dma_gathers so DVE work overlaps descriptor generation.
Everything else (identity, ones+ib rows) arrives as host inputs so the
Pool engine does nothing but the gathers + the [128,1] user-row gather.

Per-core program:
  1. dma_gather x2 (slots l<25, l>=25): pair rows -> gp [128, 50, 128].
  2. indirect gather: user_aug rows -> ue [128, 66].
  3. offset = reduce_s(w2 . gp)  (DVE broadcast multiply + strided
     reduce over the 100 half-slots).
  4. PE transpose ue -> ueT; rhs rows 0:64 = host-prepped iw.T batch,
     rows 64:66 = host [ones; ib] block.
  5. 8 matmuls [66,128]^T @ [66,512] -> PSUM -> SBUF -> DMA out.
"""

import numpy as np

N_USERS = 100000
N_ITEMS = 50000
NPAIR = N_ITEMS // 2       # 25000 paired rows; index fits int16
D = 64
D2 = 2 * D                 # 128: paired row width
BU = 1024
BI = 4096
L = 50
LH = L // 2                # 25 slots per gather chunk
NCORES = 8
UB = BU // NCORES          # 128 users per core
P = 128
K = D + 2                  # 66: augmented contraction dim
NBANK = 8                  # output column blocks of 512
NIDXH = UB * LH            # 3200 slots per gather chunk
NIDXH16 = NIDXH // 16      # 200

_cached = {}


def _build_program():
    import concourse.bacc as bacc
    import concourse.bass as bass
    import concourse.mybir as mybir
    import concourse.tile as tile

    f32 = mybir.dt.float32
    i32 = mybir.dt.int32
    i16 = mybir.dt.int16

    # Bacc (not raw Bass): its compile() legalizes sync waits for TRN2.
    nc = bacc.Bacc()

    uid = nc.dram_tensor("uid", [UB, 1], i32, kind="ExternalInput")
    idx0 = nc.dram_tensor("idx0", [P, NIDXH16], i16, kind="ExternalInput")
    idx1 = nc.dram_tensor("idx1", [P, NIDXH16], i16, kind="ExternalInput")
    w2 = nc.dram_tensor("w2", [P, 2 * L], f32, kind="ExternalInput")
    user_aug = nc.dram_tensor("user_aug", [N_USERS, K], f32, kind="ExternalInput")
    ereP = nc.dram_tensor("ereP", [NPAIR, D2], f32, kind="ExternalInput")
    iw_t = nc.dram_tensor("iw_t", [D, BI], f32, kind="ExternalInput")
    ones_ib = nc.dram_tensor("ones_ib", [2, BI], f32, kind="ExternalInput")
    ident_in = nc.dram_tensor("ident_in", [P, P], f32, kind="ExternalInput")
    rating = nc.dram_tensor("rating", [UB, BI], f32, kind="ExternalOutput")

    with tile.TileContext(nc) as tc:
        with (
            tc.tile_pool(name="sb", bufs=1) as sb,
            tc.tile_pool(name="sb_out", bufs=4) as sb_out,
            tc.tile_pool(name="ps_ue", bufs=1, space="PSUM") as ps_ue,
            tc.tile_pool(name="ps_mm", bufs=4, space="PSUM") as ps_mm,
        ):
            # --- index tiles, then the big gathers immediately ---
            i0_s = sb.tile([P, NIDXH16], i16)
            nc.sync.dma_start(out=i0_s[:], in_=idx0[:])
            i1_s = sb.tile([P, NIDXH16], i16)
            nc.sync.dma_start(out=i1_s[:], in_=idx1[:])

            gp = sb.tile([P, L * D2], f32)   # [128, 50, 128] paired rows
            for h, idx_s in ((0, i0_s), (1, i1_s)):
                nc.gpsimd.dma_gather(
                    out_ap=gp[:, h * LH * D2 : (h + 1) * LH * D2].rearrange(
                        "p (l e) -> p l e", e=D2
                    ),
                    in_ap=ereP[:],
                    idxs_ap=idx_s[:],
                    num_idxs=NIDXH,
                    num_idxs_reg=NIDXH,
                    elem_size=D2,
                    single_packet=False,
                )

            # --- user rows: ue = [uw | ub+bias | 1] ---
            uid_s = sb.tile([P, 1], i32)
            nc.sync.dma_start(out=uid_s[:], in_=uid[:])
            ue = sb.tile([P, K], f32)
            nc.gpsimd.indirect_dma_start(
                out=ue[:],
                out_offset=None,
                in_=user_aug[:],
                in_offset=bass.IndirectOffsetOnAxis(ap=uid_s[:], axis=0),
            )

            # --- other small/streaming loads ---
            w2_s = sb.tile([P, 2 * L], f32)
            nc.sync.dma_start(out=w2_s[:], in_=w2[:])
            ident = sb.tile([P, P], f32)
            nc.sync.dma_start(out=ident[:], in_=ident_in[:])
            rhs = sb.tile([K, BI], f32)
            nc.sync.dma_start(out=rhs[0:D, :], in_=iw_t[:])
            nc.sync.dma_start(out=rhs[D:K, :], in_=ones_ib[:])

            # --- offset: per-half multiply + reduce over 50 half-slots ---
            offs_h = []
            for h in range(2):
                prod = sb.tile([P, LH * D2], f32, tag=f"prod{h}")
                nc.vector.tensor_tensor(
                    out=prod[:].rearrange("p (s d) -> p s d", d=D),
                    in0=gp[:, h * LH * D2 : (h + 1) * LH * D2].rearrange(
                        "p (s d) -> p s d", d=D
                    ),
                    in1=w2_s[:, h * L : (h + 1) * L].to_broadcast([P, L, D]),
                    op=mybir.AluOpType.mult,
                )
                oh = sb.tile([P, D], f32, tag=f"offs{h}")
                nc.vector.reduce_sum(
                    out=oh[:],
                    in_=prod[:].rearrange("p (s d) -> p d s", d=D),
                    axis=mybir.AxisListType.X,
                )
                offs_h.append(oh)
            # ue[:, :D] += offs0 + offs1
            nc.vector.tensor_tensor(
                out=offs_h[0][:], in0=offs_h[0][:], in1=offs_h[1][:],
                op=mybir.AluOpType.add,
            )
            nc.vector.tensor_tensor(
                out=ue[:, 0:D], in0=ue[:, 0:D], in1=offs_h[0][:],
                op=mybir.AluOpType.add,
            )

            # --- transpose ue -> ueT [66, 128] ---
            ueT_p = ps_ue.tile([K, P], f32, space="PSUM")
            nc.tensor.transpose(out=ueT_p[:], in_=ue[:], identity=ident[:])
            ueT = sb.tile([K, P], f32)
            nc.scalar.copy(out=ueT[:], in_=ueT_p[:])

            # --- main matmuls + output ---
            for n in range(NBANK):
                mm = ps_mm.tile([P, 512], f32, space="PSUM", tag="mm")
                nc.tensor.matmul(
                    out=mm[:],
                    lhsT=ueT[:],
                    rhs=rhs[:, n * 512 : (n + 1) * 512],
                    start=True,
                    stop=True,
                )
                ot = sb_out.tile([P, 512], f32, tag="ot")
                nc.any.tensor_copy(out=ot[:], in_=mm[:])
                nc.sync.dma_start(
                    out=rating[:, n * 512 : (n + 1) * 512], in_=ot[:]
                )

    nc.finalize()
    return nc


def _get_program():
    if "nc" not in _cached:
        _cached["nc"] = _build_program()
    return _cached["nc"]


# tile[p, s] = flat_half[s*16 + p%16]: dma_gather index interleave,
# replicated across the 8 groups of 16 partitions.
_S_IDX = np.arange(NIDXH16)[None, :] * 16 + (np.arange(P) % 16)[:, None]
_IDENT = np.eye(P, dtype=np.float32)


def _prep_inputs(inputs):
    user_ids = np.asarray(inputs["user_ids"]).astype(np.int32)
    item_ids = np.asarray(inputs["item_ids"]).astype(np.int64)
    fb_indices = np.asarray(inputs["fb_indices"]).astype(np.int64)
    fb_values = np.asarray(inputs["fb_values"]).astype(np.float32)
    uw = np.asarray(inputs["user_weight"], dtype=np.float32)
    ub = np.asarray(inputs["user_bias"], dtype=np.float32).reshape(N_USERS, 1)
    iw = np.asarray(inputs["item_weight"], dtype=np.float32)
    ib = np.asarray(inputs["item_bias"], dtype=np.float32).reshape(N_ITEMS, 1)
    ire = np.ascontiguousarray(
        np.asarray(inputs["item_rating_effect_weight"], dtype=np.float32)
    )
    bias = float(np.asarray(inputs["bias"], dtype=np.float32).reshape(-1)[0])

    user_aug = np.empty((N_USERS, K), dtype=np.float32)
    user_aug[:, 0:D] = uw
    user_aug[:, D : D + 1] = ub + bias
    user_aug[:, D + 1] = 1.0

    # item batch: order known host-side; device streams it contiguously
    iw_t = np.ascontiguousarray(iw[item_ids].T)            # [64, 4096]
    ones_ib = np.empty((2, BI), dtype=np.float32)
    ones_ib[0] = 1.0
    ones_ib[1] = ib[item_ids, 0]

    ereP = ire.reshape(NPAIR, D2)                          # paired view

    in_maps = []
    for c in range(NCORES):
        sl = slice(c * UB, (c + 1) * UB)
        fbi_c = fb_indices[sl]                 # [128, 50]
        fbv_c = fb_values[sl]
        flat = fbi_c.T.reshape(-1)             # flat[l*128+p] = fbi_c[p, l]
        pair_idx = (flat // 2).astype(np.int16)
        # w2[p, 2l + parity] = fbv[p, l]; other half-slot weight 0
        w2v = np.zeros((P, 2 * L), dtype=np.float32)
        i_arr = np.arange(UB * L)
        w2v[i_arr % P, 2 * (i_arr // P) + (flat & 1)] = fbv_c.T.reshape(-1)
        in_maps.append(
            {
                "uid": user_ids[sl].reshape(UB, 1),
                "idx0": np.ascontiguousarray(pair_idx[:NIDXH][_S_IDX]),
                "idx1": np.ascontiguousarray(pair_idx[NIDXH:][_S_IDX]),
                "w2": w2v,
                "user_aug": user_aug,
                "ereP": ereP,
                "iw_t": iw_t,
                "ones_ib": ones_ib,
                "ident_in": _IDENT,
            }
        )
    return in_maps


def run(inputs, trace=False):
    """Returns (output [1024, 4096] f32, BassKernelResults)."""
    from concourse import bass_utils

    nc = _get_program()
    in_maps = _prep_inputs(inputs)
    res = bass_utils.run_bass_kernel_spmd(
        nc, in_maps, core_ids=list(range(NCORES)), trace=trace
    )
    out = np.concatenate([res.results[c]["rating"] for c in range(NCORES)], axis=0)
    return out, res


def kernel(**inputs) -> np.ndarray:
    out, _ = run(inputs, trace=False)
    return out

